# revision 14
# baseline (speedup 1.0000x reference)
"""GAE (advantage + return) reverse affine scan on 8 TRN2 NeuronCores.

Math: the reference's reversed lax.scan decomposes into two independent
first-order affine recurrences over t (run from T-1 down to 0):

    delta[i] = r[i] - v[i] + GAMMA * m[i] * v[i+1]          (pointwise)
    adv[i]   = delta[i] + (GAMMA*TAU*m[i]) * adv[i+1]        (affine scan)
    ret[i]   = (r[i] + GAMMA*(1-m[i])*nv[i]) + (GAMMA*m[i]) * ret[i+1]

Sharding: T split into 8 contiguous chunks (one per core); each core chunk
is laid out [128 partitions, F] with each partition owning a contiguous run
of F elements. Per-lane reverse scans run on the hardware tensor_tensor_scan
instruction (DVE, ~2 cycles/elem) via negative-stride access patterns,
pipelined over column-chunks. Coefficient prep runs in bf16 (DVE 2x mode)
with casts/affine ops on ScalarE and the ret-path prep on GPSIMD. Carries
across lanes/cores use per-lane affine composites (A, B): tiny DRAM-bounce
transposes + tiny scans + one 32-byte AllGather, then phase-3 rescans with
corrected initial carries.
"""

import numpy as np

GAMMA = 0.99
TAU = 0.95
P = 128
NCORES = 8
NCH = 4  # column chunks per core (pipeline granularity)

# dtype knobs (precision / speed tradeoffs)
COMPUTE_BF16 = True  # a/delta/b_ret tensors in bf16 (DVE 2x prep)
RET_PREP_ENGINE = "gpsimd"  # "gpsimd" or "vector"

_graph_cache = {}


def _build_graph(F):
    import concourse.tile as tile
    from concourse import bacc, mybir

    f32 = mybir.dt.float32
    bf16 = mybir.dt.bfloat16
    cdt = bf16 if COMPUTE_BF16 else f32
    mdt = bf16  # masks arrive as bf16 (exact 0/1)
    L = P * F
    W = F // NCH
    assert F % NCH == 0

    nc = bacc.Bacc(
        "TRN2", target_bir_lowering=False, debug=False, num_devices=NCORES
    )

    r_ext = nc.declare_dram_parameter("rewards", [L, 1], f32, isOutput=False)
    v_ext = nc.declare_dram_parameter("values", [L + 1, 1], f32, isOutput=False)
    nv_ext = nc.declare_dram_parameter("next_values", [L, 1], f32, isOutput=False)
    m_ext = nc.declare_dram_parameter("masks", [L, 1], mdt, isOutput=False)
    vb_ext = nc.declare_dram_parameter("vb", [P, 1], f32, isOutput=False)
    gt_ext = nc.declare_dram_parameter("mask_gt", [2, NCORES], f32, isOutput=False)
    le_ext = nc.declare_dram_parameter("mask_le", [2, NCORES], f32, isOutput=False)
    adv_ext = nc.declare_dram_parameter("adv", [L, 1], f32, isOutput=True)
    ret_ext = nc.declare_dram_parameter("ret", [L, 1], f32, isOutput=True)

    mult = mybir.AluOpType.mult
    add = mybir.AluOpType.add
    sub = mybir.AluOpType.subtract
    bypass = mybir.AluOpType.bypass
    Copy = mybir.ActivationFunctionType.Copy

    c_adv = GAMMA * TAU
    c_ret = GAMMA
    A_adv_F = float(np.float32(c_adv) ** F)  # may underflow to 0.0: correct
    A_ret_F = float(np.float32(c_ret) ** F)

    with tile.TileContext(nc) as tc:
        ret_eng = nc.gpsimd if RET_PREP_ENGINE == "gpsimd" else nc.vector
        with (
            tc.tile_pool(name="mio", bufs=NCH) as m_pool,
            tc.tile_pool(name="rio", bufs=3) as r_pool,
            tc.tile_pool(name="vio", bufs=NCH) as v_pool,
            tc.tile_pool(name="nio", bufs=3) as nv_pool,
            tc.tile_pool(name="cast", bufs=3) as cast_pool,
            tc.tile_pool(name="aadv", bufs=NCH) as aadv_pool,
            tc.tile_pool(name="aret", bufs=NCH) as aret_pool,
            tc.tile_pool(name="delt", bufs=NCH) as delta_pool,
            tc.tile_pool(name="bret", bufs=NCH) as bret_pool,
            tc.tile_pool(name="scr", bufs=3) as scr_pool,
            tc.tile_pool(name="y0", bufs=2) as y0_pool,
            tc.tile_pool(name="yout", bufs=4) as yout_pool,
            tc.tile_pool(name="small", bufs=1) as small,
            tc.tile_pool(name="dram", bufs=1, space="DRAM") as dram_pool,
        ):
            vb_t = small.tile([P, 1], f32)
            nc.gpsimd.dma_start(vb_t[:], vb_ext[:])
            gtile = small.tile([2, NCORES], f32)
            nc.gpsimd.dma_start(gtile[:], gt_ext[:])
            ltile = small.tile([2, NCORES], f32)
            nc.gpsimd.dma_start(ltile[:], le_ext[:])
            msums = small.tile([P, NCH], f32)

            # chunk index c runs over columns; process DESCENDING so the
            # reverse scans chain naturally (high t first).
            chunks = list(range(NCH - 1, -1, -1))
            v_c = {}
            a_adv_c, a_ret_c, delta_c, b_ret_c = {}, {}, {}, {}
            y0a_c, y0r_c = {}, {}

            # ---- DMA in + prep + phase-1 scans, chunk pipelined ---------
            for c in chunks:
                cs = slice(c * W, (c + 1) * W)
                m_t = m_pool.tile([P, W], mdt, tag="mio")
                nc.sync.dma_start(
                    m_t[:], m_ext.rearrange("(p f) o -> p (f o)", p=P)[:, cs]
                )
                r_t = r_pool.tile([P, W], f32, tag="rio")
                nc.sync.dma_start(
                    r_t[:], r_ext.rearrange("(p f) o -> p (f o)", p=P)[:, cs]
                )
                v_t = v_pool.tile([P, W], f32, tag="vio")
                nc.sync.dma_start(
                    v_t[:],
                    v_ext[0:L, :].rearrange("(p f) o -> p (f o)", p=P)[:, cs],
                )
                nv_t = nv_pool.tile([P, W], f32, tag="nio")
                nc.sync.dma_start(
                    nv_t[:], nv_ext.rearrange("(p f) o -> p (f o)", p=P)[:, cs]
                )
                v_c[c] = v_t

                # ScalarE: affine builders + casts (out dtype = cdt)
                a_adv = aadv_pool.tile([P, W], f32, tag="aadv")
                nc.scalar.activation(a_adv[:], m_t[:], Copy, scale=c_adv)
                a_ret = aret_pool.tile([P, W], f32, tag="aret")
                nc.scalar.activation(
                    a_ret[:], m_t[:], Copy, scale=c_ret,
                    accum_out=msums[:, c : c + 1],
                )
                a_adv_c[c], a_ret_c[c] = a_adv, a_ret

                rb = cast_pool.tile([P, W], cdt, tag="rb")
                nc.scalar.activation(rb[:], r_t[:], Copy)
                vcast = cast_pool.tile([P, W], cdt, tag="vcast")
                nc.scalar.activation(vcast[:], v_t[:], Copy)
                nvb = cast_pool.tile([P, W], cdt, tag="nvb")
                nc.scalar.activation(nvb[:], nv_t[:], Copy)
                vs = cast_pool.tile([P, W], cdt, tag="vs")
                nc.scalar.activation(vs[:, 0 : W - 1], v_t[:, 1:W], Copy)
                if c == NCH - 1:
                    nc.scalar.activation(vs[:, W - 1 : W], vb_t[:], Copy)
                else:
                    nc.scalar.activation(vs[:, W - 1 : W], v_c[c + 1][:, 0:1], Copy)

                # DVE: delta = (rb - vcast) + a_ret * vs
                w1 = scr_pool.tile([P, W], cdt, tag="w1")
                nc.vector.scalar_tensor_tensor(w1[:], m_t[:], c_ret, vs[:], mult, mult)
                w2 = scr_pool.tile([P, W], cdt, tag="w2")
                nc.vector.tensor_tensor(w2[:], rb[:], vcast[:], sub)
                delta = delta_pool.tile([P, W], cdt, tag="delt")
                nc.vector.tensor_tensor(delta[:], w1[:], w2[:], add)
                delta_c[c] = delta

                # ret-path prep: b_ret = r + GAMMA*nv*(1-m)
                # (Pool engine only supports plain tensor_tensor; the
                # GAMMA scale rides on ScalarE.)
                u1 = scr_pool.tile([P, W], cdt, tag="u1")
                ret_eng.tensor_tensor(u1[:], nvb[:], m_t[:], mult)
                u2 = scr_pool.tile([P, W], cdt, tag="u2")
                ret_eng.tensor_tensor(u2[:], nvb[:], u1[:], sub)
                u3 = scr_pool.tile([P, W], cdt, tag="u3")
                nc.scalar.activation(u3[:], u2[:], Copy, scale=c_ret)
                b_ret = bret_pool.tile([P, W], cdt, tag="bret")
                ret_eng.tensor_tensor(b_ret[:], rb[:], u3[:], add)
                b_ret_c[c] = b_ret

                # phase-1 scans (chained via col 0 of previous chunk's out)
                y0a = y0_pool.tile([P, W], cdt, tag="y0a")
                inita = 0.0 if c == NCH - 1 else y0a_c[c + 1][:, 0:1]
                nc.vector.tensor_tensor_scan(
                    y0a[:, ::-1], a_adv[:, ::-1], delta[:, ::-1], inita, mult, add
                )
                y0a_c[c] = y0a
                y0r = y0_pool.tile([P, W], cdt, tag="y0r")
                initr = 0.0 if c == NCH - 1 else y0r_c[c + 1][:, 0:1]
                nc.vector.tensor_tensor_scan(
                    y0r[:, ::-1], a_ret[:, ::-1], b_ret[:, ::-1], initr, mult, add
                )
                y0r_c[c] = y0r

            # ---- composites per lane: A = c^F * allm, B = y0[:, 0] ------
            msum = small.tile([P, 1], f32)
            nc.vector.tensor_reduce(msum[:], msums[:], mybir.AxisListType.X, add)
            allm = small.tile([P, 1], f32)
            # msum = GAMMA * (#ones); all-ones iff msum >= GAMMA*(F-0.5)
            nc.vector.tensor_scalar(
                allm[:], msum[:], float(GAMMA) * (F - 0.5), 0.0,
                mybir.AluOpType.is_ge, bypass,
            )
            abcols = small.tile([P, 4], f32)
            nc.vector.tensor_scalar(abcols[:, 0:1], allm[:], A_adv_F, 0.0, mult, bypass)
            nc.vector.tensor_scalar(abcols[:, 1:2], allm[:], A_ret_F, 0.0, mult, bypass)
            nc.vector.tensor_copy(abcols[:, 2:3], y0a_c[0][:, 0:1])
            nc.vector.tensor_copy(abcols[:, 3:4], y0r_c[0][:, 0:1])

            # tiny transpose via DRAM bounce + AP swap: [P,4] -> [4,P]
            dAB = dram_pool.tile([P, 4], f32)
            nc.gpsimd.dma_start(dAB[:], abcols[:])
            arowt = small.tile([2, P], f32)
            nc.gpsimd.dma_start(arowt[:], dAB[:, 0:2].rearrange("a b -> b a"))
            browt = small.tile([2, P], f32)
            nc.gpsimd.dma_start(browt[:], dAB[:, 2:4].rearrange("a b -> b a"))

            # core composite: compose lanes 127..0 applied to 0; A product
            bcomp = small.tile([2, P], f32)
            nc.vector.tensor_tensor_scan(
                bcomp[:, ::-1], arowt[:, ::-1], browt[:, ::-1], 0.0, mult, add
            )
            ones2 = small.tile([2, P], f32)
            nc.vector.memset(ones2[:], 1.0)
            acomp = small.tile([2, P], f32)
            nc.vector.tensor_tensor_scan(
                acomp[:, ::-1], arowt[:, ::-1], ones2[:, ::-1], 1.0, mult, mult
            )

            # ---- cross-core exchange: AllGather of (A_core, B_core) -----
            ccin_t = small.tile([2, 2], f32)
            nc.vector.tensor_copy(ccin_t[:, 0:1], acomp[:, 0:1])
            nc.vector.tensor_copy(ccin_t[:, 1:2], bcomp[:, 0:1])
            cc_in = dram_pool.tile([2, 2], f32)
            cc_out = dram_pool.tile([2 * NCORES, 2], f32, addr_space="Shared")
            nc.gpsimd.dma_start(cc_in[:], ccin_t[:])
            nc.gpsimd.collective_compute(
                "AllGather",
                bypass,
                replica_groups=[list(range(NCORES))],
                ins=[cc_in[:].opt()],
                outs=[cc_out[:].opt()],
            )
            Aall = small.tile([2, NCORES], f32)
            nc.gpsimd.dma_start(
                Aall[:], cc_out[:].rearrange("(j r) c -> r j c", r=2)[:, :, 0:1]
            )
            Ball = small.tile([2, NCORES], f32)
            nc.gpsimd.dma_start(
                Ball[:], cc_out[:].rearrange("(j r) c -> r j c", r=2)[:, :, 1:2]
            )

            # blend to identity for cores <= self, then compose 7..0
            tA = small.tile([2, NCORES], f32)
            nc.vector.tensor_tensor(tA[:], Aall[:], gtile[:], mult)
            tA2 = small.tile([2, NCORES], f32)
            nc.vector.tensor_tensor(tA2[:], tA[:], ltile[:], add)
            tB = small.tile([2, NCORES], f32)
            nc.vector.tensor_tensor(tB[:], Ball[:], gtile[:], mult)
            ccomp = small.tile([2, NCORES], f32)
            nc.vector.tensor_tensor_scan(
                ccomp[:, ::-1], tA2[:, ::-1], tB[:, ::-1], 0.0, mult, add
            )

            # lane-level carries: scan lanes 127..0 with core carry as init
            ls = small.tile([2, P], f32)
            nc.vector.tensor_tensor_scan(
                ls[:, ::-1], arowt[:, ::-1], browt[:, ::-1], ccomp[:, 0:1], mult, add
            )
            carry_row = small.tile([2, P], f32)
            nc.vector.tensor_copy(carry_row[:, 0 : P - 1], ls[:, 1:P])
            nc.vector.tensor_copy(carry_row[:, P - 1 : P], ccomp[:, 0:1])

            dC = dram_pool.tile([2, P], f32)
            nc.gpsimd.dma_start(dC[:], carry_row[:])
            carr = small.tile([P, 2], f32)
            nc.gpsimd.dma_start(carr[:], dC[:].rearrange("a b -> b a"))

            # ---- phase 3: rescan with corrected carries, DMA out --------
            ya_c, yr_c = {}, {}
            for c in chunks:
                cs = slice(c * W, (c + 1) * W)
                yadv = yout_pool.tile([P, W], f32, tag="ya")
                inita = carr[:, 0:1] if c == NCH - 1 else ya_c[c + 1][:, 0:1]
                nc.vector.tensor_tensor_scan(
                    yadv[:, ::-1], a_adv_c[c][:, ::-1], delta_c[c][:, ::-1],
                    inita, mult, add,
                )
                ya_c[c] = yadv
                yret = yout_pool.tile([P, W], f32, tag="yr")
                initr = carr[:, 1:2] if c == NCH - 1 else yr_c[c + 1][:, 0:1]
                nc.vector.tensor_tensor_scan(
                    yret[:, ::-1], a_ret_c[c][:, ::-1], b_ret_c[c][:, ::-1],
                    initr, mult, add,
                )
                yr_c[c] = yret
                nc.sync.dma_start(
                    adv_ext.rearrange("(p f) o -> p (f o)", p=P)[:, cs], yadv[:]
                )
                nc.sync.dma_start(
                    ret_ext.rearrange("(p f) o -> p (f o)", p=P)[:, cs], yret[:]
                )

    nc.compile()
    return nc


def get_graph(F):
    key = (F, NCH, COMPUTE_BF16, RET_PREP_ENGINE)
    if key not in _graph_cache:
        _graph_cache[key] = _build_graph(F)
    return _graph_cache[key]


def make_in_maps(rewards, values, next_values, masks):
    import ml_dtypes

    T = rewards.shape[0]
    L = T // NCORES
    F = L // P
    r = np.ascontiguousarray(rewards, dtype=np.float32).reshape(T, 1)
    nv = np.ascontiguousarray(next_values, dtype=np.float32).reshape(T, 1)
    m = np.ascontiguousarray(masks).astype(ml_dtypes.bfloat16).reshape(T, 1)
    vpad = np.empty((T + 1, 1), dtype=np.float32)
    vpad[:T] = np.asarray(values, dtype=np.float32).reshape(T, 1)
    vpad[T] = 0.0
    in_maps = []
    for k in range(NCORES):
        base = k * L
        gt = np.zeros((2, NCORES), dtype=np.float32)
        gt[:, k + 1 :] = 1.0
        vb = vpad[base + F : base + L + F : F, :][:P].copy()
        in_maps.append(
            {
                "rewards": r[base : base + L],
                "values": vpad[base : base + L + 1],
                "next_values": nv[base : base + L],
                "masks": m[base : base + L],
                "vb": vb,
                "mask_gt": gt,
                "mask_le": np.float32(1.0) - gt,
            }
        )
    return in_maps, L, F


def kernel(rewards, values, next_values, masks):
    from concourse.bass_utils import run_bass_kernel_spmd

    in_maps, L, F = make_in_maps(rewards, values, next_values, masks)
    nc = get_graph(F)
    res = run_bass_kernel_spmd(nc, in_maps, core_ids=list(range(NCORES))).results
    adv = np.concatenate([res[k]["adv"] for k in range(NCORES)], axis=0)
    ret = np.concatenate([res[k]["ret"] for k in range(NCORES)], axis=0)
    return adv, ret
